# revision 5
# baseline (speedup 1.0000x reference)
"""MoE expert-parallel kernel for Trainium2 (8 NeuronCores, 1 expert/core).

Reference computation per expert e:
    h   = relu(x_e @ W1_e)               [N, DFF]
    agg[d] += h[src[k]] for dst[k]==d    (segment-sum over NE edges)
    out = agg @ W2_e                     [N, D]

Key transformations:
  1. segment_sum is linear:  (S @ h) @ W2 == S @ (h @ W2),
     where S[d, s] = #edges s->d.  Applying W2 *before* the aggregation
     halves the cost of the aggregation matmul (D < DFF).
  2. S is built on the host from edge_index (dense count matrix) so the
     gather/scatter becomes a dense matmul on the tensor engine.
  3. Everything runs in bf16 (PSUM accumulation stays fp32).  The PE
     streams bf16 at the same 1 elem/cycle as float32r, but bf16 halves
     every DMA transfer and SBUF footprint, which makes the whole
     working set fit on-chip:
       - phases A (h = relu(W1.T @ x)) and B (m = h.T @ W2) are fused
         per 512-token slice; h never leaves SBUF (the fp32r version
         had to round-trip 64MB of hT through DRAM),
       - all 32 m chunks stay SBUF-resident for phase C (no spill),
       - the 64MB count-matrix stream becomes 32MB.
     Total per-core HBM traffic drops ~178MB -> ~64MB, so the tensor
     engine never waits on DMA.  Accuracy: ~0.3% rel err (vs 2% budget).

Device pipeline per core (expert), all on the PE back-to-back:
    for each 512-token slice (8 slices):
      phase A: h[ft] = relu( W1[:,ft].T @ x_slice )    16 x 8 MMs
      phase B: m[nt]  = h.T @ W2                        4 x 2 x 16 MMs
    phase C: out[nt] = ST[nt].T @ m                    32 x 2 x 32 MMs
W1/W2/x/ST are staged just-in-time so the first matmul starts ~4us in
and every later load hides under matmul streaming.
"""

import os

import numpy as np
from ml_dtypes import bfloat16

import concourse.bass as bass
import concourse.mybir as mybir
import concourse.tile as tile
from concourse import bacc
from concourse.bass_utils import run_bass_kernel_spmd

E, N, D, DFF = 8, 4096, 1024, 2048
P = 128
NT = N // P     # 32  n tiles (output rows / m chunks)
DC = D // P     # 8   d chunks (K for phase A)
FT = DFF // P   # 16  f tiles  (K for phase B)
DS = D // 512   # 2   d slices of 512
NS = N // 512   # 8   n slices of 512
LT = 512 // P   # 4   n tiles per n slice

F32 = mybir.dt.float32
BF16 = mybir.dt.bfloat16
RELU = mybir.ActivationFunctionType.Relu

_cache = {}


def _emit(nc, tc):
    xT = nc.dram_tensor("xT", [D, N], BF16, kind="ExternalInput")
    # W1H[ft, p, dc, f'] = W1[dc*128 + p, ft*128 + f']  (one 256KB chunk
    # per f-tile so the first matmul group waits on ~0.4MB of DMA)
    W1H = nc.dram_tensor("W1H", [FT, P, DC, P], BF16, kind="ExternalInput")
    # W2H[fc, p, d] = W2[fc*128 + p, d]
    W2H = nc.dram_tensor("W2H", [FT, P, D], BF16, kind="ExternalInput")
    # ST[nt, p, sc, n'] = S_T[sc*128 + p, nt*128 + n']
    ST = nc.dram_tensor("ST", [NT, P, NT, P], BF16, kind="ExternalInput")
    out = nc.dram_tensor("out", [N, D], F32, kind="ExternalOutput")

    xTr = xT.rearrange("(dc p) n -> p dc n", p=P)

    # m chunks live across phases B and C
    with tc.tile_pool(name="mp", bufs=1) as mp:
        msb = [None] * NT

        with tc.tile_pool(name="w1p", bufs=1) as w1p, \
             tc.tile_pool(name="w2p", bufs=1) as w2p, \
             tc.tile_pool(name="xp", bufs=2) as xp, \
             tc.tile_pool(name="hp", bufs=2) as hp, \
             tc.tile_pool(name="psA", bufs=3, space="PSUM") as psA, \
             tc.tile_pool(name="psB", bufs=3, space="PSUM") as psB:
            w1sb = w1p.tile([P, FT, DC, P], BF16, name="w1sb")
            w2sb = w2p.tile([P, FT, D], BF16, name="w2sb")
            xsbs = [
                xp.tile([P, DC, 512], BF16, tag="xsb", name=f"xsb{ns}")
                for ns in range(NS)
            ]
            # startup: first W1 chunk, then x slice 0 dc-by-dc so the
            # first accumulation group starts after ~0.4MB of DMA
            nc.sync.dma_start(out=w1sb[:, 0], in_=W1H[0])
            for dc in range(DC):
                nc.sync.dma_start(
                    out=xsbs[0][:, dc], in_=xTr[:, dc, 0:512]
                )
            # x slice 1 lands during the ~12us kernel preamble + first
            # groups, so slice-0's W1/W2 staging never queues behind it
            nc.sync.dma_start(out=xsbs[1][:], in_=xTr[:, :, 512:1024])
            for ns in range(NS):
                xsb = xsbs[ns]
                # ---------- phase A: h = relu(W1.T @ x) ----------
                hsb = [None] * FT
                for ft in range(FT):
                    pt = psA.tile([P, 512], F32, name="ptA")
                    for dc in range(DC):
                        nc.tensor.matmul(
                            out=pt[:],
                            lhsT=w1sb[:, ft, dc, :],
                            rhs=xsb[:, dc, :],
                            start=(dc == 0),
                            stop=(dc == DC - 1),
                        )
                    hsb[ft] = hp.tile(
                        [P, 512], BF16, tag=f"h{ft}", name=f"hsb{ft}"
                    )
                    nc.scalar.activation(out=hsb[ft][:], in_=pt[:], func=RELU)
                    if ns == 0:
                        # JIT-stage the next W1 chunk + one W2 chunk
                        # behind this group (W2 fully resident by B(0))
                        if ft + 1 < FT:
                            nc.sync.dma_start(
                                out=w1sb[:, ft + 1], in_=W1H[ft + 1]
                            )
                        nc.sync.dma_start(out=w2sb[:, ft], in_=W2H[ft])
                    if ft == 0 and 1 <= ns < NS - 1:
                        # prefetch next x slice (slice 1 was prefetched
                        # at startup)
                        nc.sync.dma_start(
                            out=xsbs[ns + 1][:],
                            in_=xTr[:, :, (ns + 1) * 512 : (ns + 2) * 512],
                        )
                # ---------- phase B: m = h.T @ W2 ----------
                for lt in range(LT):
                    nt = ns * LT + lt
                    msb[nt] = mp.tile(
                        [P, D], BF16, tag=f"m{nt}", name=f"msb{nt}"
                    )
                    ptd = [
                        psB.tile([P, 512], F32, name="ptB") for _ in range(DS)
                    ]
                    for fc in range(FT):
                        lhs = hsb[fc][:, lt * P : (lt + 1) * P]
                        for ds in range(DS):
                            nc.tensor.matmul(
                                out=ptd[ds][:],
                                lhsT=lhs,
                                rhs=w2sb[:, fc, ds * 512 : (ds + 1) * 512],
                                start=(fc == 0),
                                stop=(fc == FT - 1),
                            )
                    for ds in range(DS):
                        nc.vector.tensor_copy(
                            out=msb[nt][:, ds * 512 : (ds + 1) * 512],
                            in_=ptd[ds][:],
                        )

        # ---------- phase C: out = ST.T @ m ----------
        with tc.tile_pool(name="stp", bufs=2) as stp, \
             tc.tile_pool(name="op", bufs=3) as op, \
             tc.tile_pool(name="psC", bufs=4, space="PSUM") as psC:
            stsbs = [
                stp.tile([P, NT, P], BF16, tag="stsb", name=f"stsb{nt}")
                for nt in range(NT)
            ]
            nc.sync.dma_start(out=stsbs[0][:], in_=ST[0])
            for nt in range(NT):
                stsb = stsbs[nt]
                if nt + 1 < NT:
                    nc.sync.dma_start(out=stsbs[nt + 1][:], in_=ST[nt + 1])
                ptd = [psC.tile([P, 512], F32, name="ptC") for _ in range(DS)]
                for sc in range(NT):
                    for ds in range(DS):
                        nc.tensor.matmul(
                            out=ptd[ds][:],
                            lhsT=stsb[:, sc, :],
                            rhs=msb[sc][:, ds * 512 : (ds + 1) * 512],
                            start=(sc == 0),
                            stop=(sc == NT - 1),
                        )
                for ds in range(DS):
                    osb = op.tile([P, 512], F32, name="osb")
                    nc.vector.tensor_copy(out=osb[:], in_=ptd[ds][:])
                    nc.sync.dma_start(
                        out=out[
                            nt * P : (nt + 1) * P, ds * 512 : (ds + 1) * 512
                        ],
                        in_=osb[:],
                    )


def _build():
    nc = bacc.Bacc()
    with tile.TileContext(nc) as tc:
        _emit(nc, tc)
    nc.compile()
    return nc


def kernel(x, W1, W2, edge_index):
    x = np.asarray(x, dtype=np.float32)
    W1 = np.asarray(W1, dtype=np.float32)
    W2 = np.asarray(W2, dtype=np.float32)
    edge_index = np.asarray(edge_index)

    # S_T[s, d] = #edges with src==s and dst==d  (so out = S_T.T @ m)
    src = edge_index[0].astype(np.int64)
    dst = edge_index[1].astype(np.int64)
    counts = np.bincount(src * N + dst, minlength=N * N)
    S_T = counts.reshape(N, N).astype(bfloat16)
    # host tiling for contiguous phase-C DMA: [nt, p, sc, n']
    STH = np.ascontiguousarray(S_T.reshape(NT, P, NT, P).transpose(2, 1, 0, 3))

    if "nc" not in _cache:
        _cache["nc"] = _build()
    nc = _cache["nc"]

    in_maps = []
    for e in range(E):
        # W1H[ft, p, dc, f'] = W1[e, dc*128+p, ft*128+f']
        W1H = np.ascontiguousarray(
            W1[e].reshape(DC, P, FT, P).transpose(2, 1, 0, 3).astype(bfloat16)
        )
        in_maps.append(
            {
                "xT": np.ascontiguousarray(x[e].T.astype(bfloat16)),
                "W1H": W1H,
                "W2H": np.ascontiguousarray(
                    W2[e].reshape(FT, P, D).astype(bfloat16)
                ),
                "ST": STH,
            }
        )

    trace = bool(int(os.environ.get("PROBLEM_TRACE", "0")))
    res = run_bass_kernel_spmd(nc, in_maps, core_ids=list(range(E)), trace=trace)
    _cache["last_results"] = res
    return np.stack([res.results[e]["out"] for e in range(E)]).astype(np.float32)
